# revision 35
# baseline (speedup 1.0000x reference)
"""Trainium2 Bass kernel for the ActorCritic ragged-sequence problem.

Strategy (v8: tight ragged packing + static tail)
-------------------------------------------------
Data-parallel over batch B=64 across 8 NeuronCores, computing pair scores
only for each row's valid prefix (lengths are ragged in [2, S=1024]).
Rows are assigned to cores by cardinality-constrained LPT (+local swaps),
each core packs its 8 rows back-to-back into a W = NSL*512 column strip
(NSL = 9 for the reference lengths vs 16 dense: ~1.8x less matmul work),
with its longest row packed last so the final 512 columns belong to one
row on every core.

Per core the pair-MLP h = relu(x_t @ W1a + x_{t+1} @ W1b + b1p),
score = w2p.h runs as weight-stationary fp8 DoubleRow matmuls (K=256 per
instruction, FD=512) over the packed strip; the +1 shift of the pair's
second element is a one-element slice offset into each 528-wide window.
Scores land in a packed [1, W] SBUF row; slices 0..NSL-2 stream to a
small Internal-DRAM scratch, from which ONE indirect element-granular
gather (SWDGE, per-core chunk-offset table as data) pulls per-row-aligned
128-col chunks onto partitions 0..59 - it fires under the last slice's
compute.  The final 512 columns bypass DRAM entirely via a static
SBUF->SBUF DMA into partitions 60-63.  A [72, 128] exp/entropy pass
(rows 64-71 hold the symbol-head logits, moved there by a tiny DMA)
produces per-partition partials which two small matmuls against 0/1
selection matrices (per-core data) combine into per-row Z/S2/logp;
masks at -1e30 kill row-boundary junk and padding.  If a pathological
length set breaks the static-tail precondition, a second indirect gather
covers the last slices instead (both variants compiled on demand, cached
by (NSL, NB, static)).

The symbol head runs in fp8 (weights prescaled by powers of two, undone
exactly on chip), the critic in bf16; both are emitted mid-loop so their
matmuls hide in the PE stream.  DMA queues: sync HWDGE carries the
x-window stream + main weights (merged transfers, issue order = arrival
order via order-only dep edges); gpsimd SWDGE carries aux/late weights so
the scalar engine stays free for activations.  76 dummy warm-up matmuls
lift the PE HAM clock gate during the initial DMA fill.  Host work is
pure indexing / layout / quantization - no FLOPs moved off-device.

Measured on trn2 (8 cores): ~97-102 us HW exec (baseline 185 us), rel
err ~2e-3 vs the fp32 reference (gate 2e-2).
"""

import os
import numpy as np

B, S, E, A = 64, 1024, 512, 128
NCORES = 8
BC = B // NCORES          # batch rows per core (= slots per core)
H = 2 * E                 # pair-MLP hidden dim
RS = 512                  # matmul moving free dim per slice
KT = E // 128             # 4 k-tiles over the E features
K2 = KT // 2              # 2 fp8 DoubleRow k-tiles (K=256 each)
CT = H // 128             # 8 chan tiles of the hidden dim
XW = 512 + 16             # padded window width (512 cols + boundary + pad)
VCT = E // 128            # chan tiles of the critic hidden dim

TRACE = os.environ.get("K_TRACE", "1") == "1"

LAST_EXEC_NS = None
_CACHED = {}

_LDWOPT = os.environ.get("K_LDWOPT", "0") == "1"
_PATCHED = False

FP8_WSCALE = 32.0    # power-of-two prescale keeping fp8 W1p/W1s mid-range
FP8_W2SCALE = 256.0  # prescale for w2p/W2s; undone exactly on chip


def _patch_walrus_flags():
    """Re-enable walrus LDWEIGHTS dedup (repeated stationary operands) for
    this process's compiles."""
    global _PATCHED
    if _PATCHED or not _LDWOPT:
        return
    import concourse.bass_utils as _bu

    _orig = _bu.run_command

    def _rc(argv, **kw):
        argv = [
            "--enable-ldw-opt=true" if a == "--enable-ldw-opt=false" else a
            for a in argv
        ]
        return _orig(argv, **kw)

    _bu.run_command = _rc
    _PATCHED = True


def _plan(lengths):
    """LPT assignment of the 64 rows to 8 cores (tight packing): returns
    (cores: list of 8 row-lists, NSL)."""
    ln = np.asarray(lengths).astype(np.int64)
    order = np.argsort(-ln, kind="stable")
    cores = [[] for _ in range(NCORES)]
    sums = np.zeros(NCORES, np.int64)
    for g in order:
        open_c = [c for c in range(NCORES) if len(cores[c]) < BC]
        c = min(open_c, key=lambda c: sums[c])
        cores[c].append(int(g))
        sums[c] += ln[g]
    # local improvement: swap rows between the fullest core and others
    for _ in range(200):
        hi = int(np.argmax(sums))
        best = None
        for c in range(NCORES):
            if c == hi:
                continue
            for i, gi in enumerate(cores[hi]):
                for k, gk in enumerate(cores[c]):
                    delta = int(ln[gi] - ln[gk])
                    if delta <= 0:
                        continue
                    new_hi = sums[hi] - delta
                    new_c = sums[c] + delta
                    new_max = max(new_hi, new_c)
                    if new_max < sums[hi] and (best is None or new_max < best[0]):
                        best = (new_max, c, i, k, delta)
        if best is None:
            break
        _, c, i, k, delta = best
        cores[hi][i], cores[c][k] = cores[c][k], cores[hi][i]
        sums[hi] -= delta
        sums[c] += delta
    NSL = (int(sums.max()) + RS - 1) // RS
    # put each core's longest row last (enables the static tail chunks)
    out = []
    for cs in cores:
        cs = list(map(int, cs))
        jmax = max(range(BC), key=lambda j: ln[cs[j]])
        cs.append(cs.pop(jmax))
        out.append(cs)
    return out, NSL


def _cfg(NSL, NB, ST):
    return dict(NSL=NSL, W=NSL * RS, NB=NB, ST=ST)


def _build(cfg):
    import concourse.tile as tile
    from concourse import bacc, mybir
    from concourse.tile_rust import add_dep_helper

    _patch_walrus_flags()

    F32 = mybir.dt.float32
    BF16 = mybir.dt.bfloat16
    F8 = mybir.dt.float8e4
    AF = mybir.ActivationFunctionType
    OP = mybir.AluOpType
    AX = mybir.AxisListType
    DR = mybir.MatmulPerfMode.DoubleRow

    NSL, W, NB, ST = cfg["NSL"], cfg["W"], cfg["NB"], cfg["ST"]
    NA0 = 60 if ST else 64 - NB
    NBPAD = NB
    BS = (NSL - 2) * RS   # spkB DRAM scratch covers packed cols [BS, W+128)

    nc = bacc.Bacc("TRN2", target_bir_lowering=False, debug=False)

    # ---- DRAM parameters -------------------------------------------------
    # packed, pair-interleaved fp8 states: [k2, window, part, plane, col]
    xt_d = nc.dram_tensor("xt8", [K2, NSL, 128, 2, XW], F8, kind="ExternalInput")
    # ct-major pair-MLP weights in 4 ct-pair chunks: [q, p, ct', ab, k2, jj, m]
    wab_d = nc.dram_tensor("wab8", [4, 128, 2, 2, K2, 2, 128], F8,
                           kind="ExternalInput")
    w2p_d = nc.dram_tensor("w2p8", [128, 2, 16], F8, kind="ExternalInput")
    mask_d = nc.dram_tensor("mask2", [64, 128], F32, kind="ExternalInput")
    idxA_d = nc.dram_tensor("gidxA", [64, 1], mybir.dt.int32, kind="ExternalInput")
    idxB_d = None
    if not ST:
        idxB_d = nc.dram_tensor("gidxB", [NBPAD, 1], mybir.dt.int32,
                                kind="ExternalInput")
    spkA_d = nc.dram_tensor("spkA", [(NSL - 1) * RS + (128 if ST else 0), 1],
                            F32, kind="Internal")
    spkB_d = None
    if not ST:
        spkB_d = nc.dram_tensor("spkB", [2 * RS + 128, 1], F32,
                                kind="Internal")
    # merged aux: fp32 [b1p | b1s | bc1 | bc2]
    aux32_d = nc.dram_tensor("aux32", [128, 2 * CT + KT + 1], F32,
                             kind="ExternalInput")
    # merged aux: fp32 [oh_all | sel]
    aux32b_d = nc.dram_tensor("aux32b", [72, 128 + 2 * BC], F32,
                              kind="ExternalInput")
    # merged aux: bf16 [wc2 | clst | b2s,ones on partition 0]
    auxbf_d = nc.dram_tensor("auxbf", [128, KT + KT * BC + A + BC], BF16,
                             kind="ExternalInput")
    e12_d = nc.dram_tensor("e12t", [128, CT * BC], F8, kind="ExternalInput")
    wsw2s_d = nc.dram_tensor("wsw2s8", [128, CT * H + CT * A], F8,
                             kind="ExternalInput")
    wc1_d = nc.dram_tensor("wc1", [128, KT * E], BF16, kind="ExternalInput")
    out_d = nc.dram_tensor("out", [BC, 5], F32, kind="ExternalOutput")

    with tile.TileContext(nc) as tc:
        with (
            tc.tile_pool(name="weights", bufs=1) as wpool,
            tc.tile_pool(name="hbuf", bufs=2) as hpool,
            tc.tile_pool(name="small", bufs=1) as spool,
            tc.tile_pool(name="psmain", bufs=2, space="PSUM") as psmain,
            tc.tile_pool(name="pssc", bufs=2, space="PSUM") as pssc,
            tc.tile_pool(name="ps3", bufs=2, space="PSUM") as ps3,
        ):
            # ---- PE warm-up: dummy matmuls during the initial DMA fill ---
            wtmp = spool.tile([128, 64], F8, name="wtmp")
            nc.vector.memset(wtmp[:], 0.0)
            for i in range(76):
                pw = psmain.tile([64, 64], F32, name="pw", tag=f"ps{i % 2}")
                nc.tensor.matmul(pw[:], wtmp[:], wtmp[:], start=True, stop=True)

            # ---- sync HWDGE queue: x windows + main weights --------------
            xbf = {}
            sync_dmas = []

            def qsync(dst, src):
                dma = nc.sync.dma_start(dst, src)
                if sync_dmas:
                    add_dep_helper(dma.ins, sync_dmas[-1].ins, False,
                                   "sync dma issue order")
                sync_dmas.append(dma)
                return dma

            wab_sb = [wpool.tile([128, 2, 2, K2, 2, 128], F8,
                                 name=f"wabq{q}") for q in range(4)]

            def xwin(s):
                for k2 in range(K2):
                    t = wpool.tile([128, 2, XW], F8, name=f"x8_{k2}_{s}")
                    xbf[(k2, s)] = t
                    qsync(t[:], xt_d[k2, s, :, :, :])

            # interleave the first windows with the weight chunks so slice 0
            # can start as soon as window 0 + the first ct-pair weights land
            xwin(0)
            qsync(wab_sb[0][:], wab_d[0, :, :, :, :, :, :])
            qsync(wab_sb[1][:], wab_d[1, :, :, :, :, :, :])
            if NSL > 1:
                xwin(1)
            qsync(wab_sb[2][:], wab_d[2, :, :, :, :, :, :])
            qsync(wab_sb[3][:], wab_d[3, :, :, :, :, :, :])
            for s in range(2, NSL):
                xwin(s)

            def wab_ap(ct, ab, k2):
                return wab_sb[ct // 2][:, ct % 2, ab, k2, :, :]

            # ---- gpsimd SWDGE queue: aux + symbol/critic weights ---------
            gp_dmas = []

            def qgp(dst, src):
                dma = nc.gpsimd.dma_start(dst, src)
                if gp_dmas:
                    add_dep_helper(dma.ins, gp_dmas[-1].ins, False,
                                   "gpsimd dma issue order")
                gp_dmas.append(dma)
                return dma

            aux32_sb = wpool.tile([128, 2 * CT + KT + 1], F32, name="aux32")
            qgp(aux32_sb[:], aux32_d[:, :])
            w2p_sb = wpool.tile([128, 2, 16], F8, name="w2p")
            qgp(w2p_sb[:], w2p_d[:, :, :])
            sm_all = spool.tile([72, 128], F32, name="small")
            mask_sb = wpool.tile([64, 128], F32, name="mask2")
            qgp(mask_sb[:], mask_d[:, :])
            idxA_sb = wpool.tile([64, 1], mybir.dt.int32, name="gidxA")
            qgp(idxA_sb[:], idxA_d[:, :])
            if not ST:
                idxB_sb = wpool.tile([NBPAD, 1], mybir.dt.int32, name="gidxB")
                qgp(idxB_sb[:], idxB_d[:, :])
            e12_sb = wpool.tile([128, CT * BC], F8, name="e12")
            qgp(e12_sb[:], e12_d[:, :])
            aux32b_sb = wpool.tile([72, 128 + 2 * BC], F32, name="aux32b")
            qgp(aux32b_sb[:], aux32b_d[:, :])
            auxbf_sb = wpool.tile([128, KT + KT * BC + A + BC], BF16,
                                  name="auxbf")
            qgp(auxbf_sb[:], auxbf_d[:, :])
            wsw2s_sb = wpool.tile([128, CT * H + CT * A], F8, name="wsw2s")
            qgp(wsw2s_sb[:], wsw2s_d[:, :])
            wc1_sb = wpool.tile([128, KT * E], BF16, name="wc1")
            qgp(wc1_sb[:], wc1_d[:, :])

            # ---- packed score row + tail tiles ---------------------------
            scores_pk = spool.tile([1, W], F32, name="scpk")
            zt = spool.tile([1, 128], F32, name="zt")
            nc.vector.memset(zt[:], 0.0)
            if ST:
                nc.sync.dma_start(
                    spkA_d[(NSL - 1) * RS : (NSL - 1) * RS + 128, 0:1], zt[:]
                )
            else:
                nc.sync.dma_start(spkB_d[2 * RS : 2 * RS + 128, 0:1], zt[:])
            smy_tmp = spool.tile([BC, A], F32, name="smyt")
            outbuf = spool.tile([BC, 5], F32, name="outbuf")

            # preload the Exp activation table off the critical path
            dume = spool.tile([1, 16], F32, name="dume")
            nc.scalar.activation(dume[:], wtmp[0:1, 0:16], AF.Exp)

            def emit_symcrit():
                # symbol head (fp8, scaled by 32/256, undone on copy)
                sh_sb = [spool.tile([128, BC], F8, name=f"sh{ct}")
                         for ct in range(CT)]
                for ct in range(CT):
                    p3 = ps3.tile([128, BC], F32, name="p3", tag="p3")
                    for k in range(CT):
                        nc.tensor.matmul(
                            p3[:],
                            wsw2s_sb[:, k * H + ct * 128 : k * H + (ct + 1) * 128],
                            e12_sb[:, k * BC : (k + 1) * BC],
                            start=(k == 0),
                            stop=(k == CT - 1),
                        )
                    nc.scalar.activation(
                        sh_sb[ct][:], p3[:], AF.Relu,
                        bias=aux32_sb[:, CT + ct : CT + ct + 1],
                    )
                psl = ps3.tile([BC, A], F32, name="psl", tag="p3")
                for ct in range(CT):
                    nc.tensor.matmul(
                        psl[:], sh_sb[ct][:],
                        wsw2s_sb[:, CT * H + ct * A : CT * H + (ct + 1) * A],
                        start=(ct == 0), stop=False,
                    )
                nc.tensor.matmul(
                    psl[:],
                    auxbf_sb[0:1, KT + KT * BC + A : KT + KT * BC + A + BC],
                    auxbf_sb[0:1, KT + KT * BC : KT + KT * BC + A],
                    start=False, stop=True,
                )
                # rescale at partitions 0-7, then DMA to partitions 64-71
                # (engine ops are partition-locked, DMAs are not)
                nc.scalar.activation(
                    smy_tmp[:], psl[:], AF.Copy, bias=0.0, scale=1.0 / 8192.0
                )
                nc.sync.dma_start(sm_all[64:72, :], smy_tmp[:])

                # critic (bf16)
                hc_sb = [spool.tile([128, BC], BF16, name=f"hc{ct}")
                         for ct in range(VCT)]
                for ct in range(VCT):
                    pc = ps3.tile([128, BC], F32, name="pc", tag="p3")
                    for k in range(KT):
                        nc.tensor.matmul(
                            pc[:],
                            wc1_sb[:, k * E + ct * 128 : k * E + (ct + 1) * 128],
                            auxbf_sb[:, KT + k * BC : KT + (k + 1) * BC],
                            start=(k == 0),
                            stop=(k == KT - 1),
                        )
                    nc.scalar.activation(
                        hc_sb[ct][:], pc[:], AF.Relu,
                        bias=aux32_sb[:, 2 * CT + ct : 2 * CT + ct + 1],
                    )
                pv = ps3.tile([BC, 1], F32, name="pv", tag="p3")
                for ct in range(VCT):
                    nc.tensor.matmul(
                        pv[:], hc_sb[ct][:], auxbf_sb[:, ct : ct + 1],
                        start=(ct == 0), stop=(ct == VCT - 1),
                    )
                nc.vector.tensor_add(outbuf[:, 2:3], pv[:],
                                     aux32_sb[0:BC, 2 * CT + KT : 2 * CT + KT + 1])  # val

            SYM_AT = min(2, NSL - 1)

            # ---- main pair-MLP over packed slices ------------------------
            for s in range(NSL):
                FD = RS
                hs = {}
                for ct in range(CT):
                    ps = psmain.tile([128, RS], F32, name=f"ps{s}_{ct}",
                                     tag=f"ps{s % 2}")
                    for wi in range(4):
                        ab, k2 = divmod(wi, K2)
                        nc.tensor.matmul(
                            ps[:, :FD],
                            wab_ap(ct, ab, k2),
                            xbf[(k2, s)][:, :, ab : ab + FD],
                            start=(wi == 0),
                            stop=(wi == 3),
                            perf_mode=DR,
                        )
                    m, jj = divmod(ct, 2)
                    if (s, m) not in hs:
                        hs[(s, m)] = hpool.tile([128, 2, RS], F8,
                                                name=f"h8_{m}", tag=f"h8_{m}")
                    plane = hs[(s, m)][:, jj, :FD]
                    # split bias+relu ~2:1 DVE:ACT so both stay in PE shadow
                    if (s * CT + ct) % 3 == 2:
                        nc.scalar.activation(
                            plane, ps[:, :FD], AF.Relu,
                            bias=aux32_sb[:, ct : ct + 1],
                        )
                    else:
                        nc.vector.tensor_scalar(
                            plane, ps[:, :FD], aux32_sb[:, ct : ct + 1], 0.0,
                            OP.add, OP.max,
                        )
                psd = pssc.tile([1, RS], F32, name="psd", tag="psd")
                for m in range(CT // 2):
                    nc.tensor.matmul(
                        psd[:, :FD],
                        w2p_sb[:, :, m : m + 1],
                        hs[(s, m)][:, :, :FD],
                        start=(m == 0),
                        stop=(m == CT // 2 - 1),
                        perf_mode=DR,
                    )
                nc.scalar.activation(
                    scores_pk[0:1, s * RS : s * RS + FD], psd[:, :FD],
                    AF.Copy, bias=0.0, scale=1.0 / 8192.0,
                )
                strip = scores_pk[0:1, s * RS : s * RS + FD]
                if s <= NSL - 2:
                    nc.sync.dma_start(
                        spkA_d[s * RS : s * RS + FD, 0:1], strip
                    )
                if not ST and s >= NSL - 2:
                    nc.sync.dma_start(
                        spkB_d[s * RS - BS : s * RS - BS + FD, 0:1], strip
                    )
                if s == SYM_AT:
                    # interleave the (tiny) symbol head + critic here: their
                    # weights have landed by now and the PE queue is in-order
                    emit_symcrit()

            # ---- accumulate-scatter packed scores onto the mask-prefilled
            # [64, 128] chunk layout (slot j -> partitions 8j..8j+chunks) --
            # two indirect element-granular gathers pull the per-row-aligned
            # chunks from the DRAM score scratch: rows [0:NA0] depend only on
            # slices <= NSL-2 (spkA), rows [NA0:64] on the final slices
            # (spkB).  Chunk indices are per-core DATA (tight LPT packing).
            from concourse.bass import IndirectOffsetOnAxis

            scr2 = spool.tile([64, 128], F32, name="scr2")
            nc.gpsimd.indirect_dma_start(
                scr2[0:NA0, :], None,
                spkA_d[:, :],
                IndirectOffsetOnAxis(ap=idxA_sb[0:NA0, 0:1], axis=0),
            )
            if ST:
                # last 512 packed cols are the longest row's tail on every
                # core: a single static SBUF->SBUF chunk DMA, no DRAM hop
                nc.sync.dma_start(
                    scr2[60:64, :], scores_pk[0:1, W - 512 : W]
                )
            else:
                nc.gpsimd.indirect_dma_start(
                    scr2[64 - NBPAD : 64, :], None,
                    spkB_d[:, :],
                    IndirectOffsetOnAxis(ap=idxB_sb[:, :], axis=0),
                )
            nc.vector.tensor_add(sm_all[0:64, :], scr2[:], mask_sb[:])

            # ---- softmax statistics over [72, 128] -----------------------
            pexp = spool.tile([72, 128], F32, name="pexp")
            pcols = spool.tile([72, 3], F32, name="pcols")
            nc.scalar.activation(
                pexp[:], sm_all[:], AF.Exp, accum_out=pcols[:, 0:1]
            )
            tmp = spool.tile([72, 128], F32, name="tmpa")
            nc.vector.tensor_mul(tmp[:], sm_all[:], aux32b_sb[:, 0:128])
            nc.vector.tensor_reduce(pcols[:, 2:3], tmp[:], axis=AX.X, op=OP.add)
            p2 = spool.tile([72, 128], F32, name="p2")
            nc.vector.tensor_mul(p2[:], pexp[:], sm_all[:])
            nc.vector.tensor_reduce(pcols[:, 1:2], p2[:], axis=AX.X, op=OP.add)

            # ---- per-row combine via tiny matmuls (psB's operands both
            # live at base partition 64 so the contraction indices align) --
            psA = ps3.tile([BC, 3], F32, name="psA", tag="p3")
            nc.tensor.matmul(psA[:], aux32b_sb[0:64, 128 : 128 + BC], pcols[0:64, :],
                             start=True, stop=True)
            psB = ps3.tile([BC, 3], F32, name="psB", tag="p3")
            nc.tensor.matmul(psB[:], aux32b_sb[64:72, 128 + BC : 128 + 2 * BC],
                             pcols[64:72, :], start=True, stop=True)

            lseA = spool.tile([BC, 1], F32, name="lseA")
            lseB = spool.tile([BC, 1], F32, name="lseB")
            nc.scalar.activation(lseA[:], psA[:, 0:1], AF.Ln)
            nc.scalar.activation(lseB[:], psB[:, 0:1], AF.Ln)
            rzA = spool.tile([BC, 1], F32, name="rzA")
            rzB = spool.tile([BC, 1], F32, name="rzB")
            nc.vector.reciprocal(rzA[:], psA[:, 0:1])
            nc.vector.reciprocal(rzB[:], psB[:, 0:1])
            s2zA = spool.tile([BC, 1], F32, name="s2zA")
            s2zB = spool.tile([BC, 1], F32, name="s2zB")
            nc.vector.tensor_mul(s2zA[:], psA[:, 1:2], rzA[:])
            nc.vector.tensor_mul(s2zB[:], psB[:, 1:2], rzB[:])
            nc.vector.tensor_sub(outbuf[:, 0:1], psA[:, 2:3], lseA[:])  # logp_pos
            nc.vector.tensor_sub(outbuf[:, 1:2], psB[:, 2:3], lseB[:])  # logp_sym
            nc.vector.tensor_sub(outbuf[:, 3:4], lseA[:], s2zA[:])      # ent_pos
            nc.vector.tensor_sub(outbuf[:, 4:5], lseB[:], s2zB[:])      # ent_sym

            nc.sync.dma_start(out_d[:, :], outbuf[:])

    nc.compile()
    return nc


def _to_cd(arr):
    import ml_dtypes

    return np.ascontiguousarray(arr).astype(ml_dtypes.bfloat16)


def _to_f8(arr):
    import ml_dtypes

    return np.ascontiguousarray(arr).astype(ml_dtypes.float8_e4m3)


def _ntff_profile_via_ctypes(so_path):
    """(dir, device_ids) -> contextmanager hook driving NTFF profiling via
    ctypes calls into the axon PJRT .so (mirrors the boot-side helper)."""
    import contextlib
    import ctypes
    import sys

    try:
        lib = ctypes.CDLL(so_path)
    except OSError:
        return None
    if not hasattr(lib, "axon_start_nrt_profile"):
        return None
    lib.axon_start_nrt_profile.argtypes = [
        ctypes.POINTER(ctypes.c_int64),
        ctypes.c_size_t,
    ]
    lib.axon_start_nrt_profile.restype = ctypes.c_int64
    lib.axon_stop_nrt_profile.argtypes = [ctypes.c_char_p]
    lib.axon_stop_nrt_profile.restype = ctypes.c_int64

    @contextlib.contextmanager
    def _hook(output_dir, device_ids):
        import jax

        jax.devices()
        if device_ids:
            ids = (ctypes.c_int64 * len(device_ids))(*device_ids)
            rc = lib.axon_start_nrt_profile(ids, len(device_ids))
        else:
            rc = lib.axon_start_nrt_profile(None, 0)
        if rc != 0:
            raise RuntimeError(f"axon_start_nrt_profile rc={rc}")
        try:
            yield
        finally:
            n = lib.axon_stop_nrt_profile(str(output_dir).encode())
            if n < 0:
                raise RuntimeError(f"axon_stop_nrt_profile rc={n}")
            print(f"profile: {n} file(s) written to {output_dir}", file=sys.stderr)

    return _hook


def _ensure_axon_hooks():
    """bass_utils imports antenv.axon_hooks unconditionally when tracing
    under axon; provide a registry (with the real ctypes-backed NTFF hook
    when the axon .so is present) if the image lacks it."""
    try:
        import antenv.axon_hooks as _h  # noqa: F401
        if _h.get_axon_ntff_profile_hook() is None:
            hook = _ntff_profile_via_ctypes("/opt/axon/libaxon_pjrt.so")
            if hook is not None:
                _h.set_axon_ntff_profile_hook(hook)
        return
    except ImportError:
        pass
    import sys
    import types

    try:
        import antenv
    except ImportError:
        return
    mod = types.ModuleType("antenv.axon_hooks")
    mod._hook = _ntff_profile_via_ctypes("/opt/axon/libaxon_pjrt.so")
    mod.set_axon_ntff_profile_hook = lambda h: setattr(mod, "_hook", h)
    mod.get_axon_ntff_profile_hook = lambda: mod._hook
    sys.modules["antenv.axon_hooks"] = mod
    antenv.axon_hooks = mod


def kernel(**inputs):
    global LAST_EXEC_NS
    import ml_dtypes
    from concourse.bass_utils import run_bass_kernel_spmd

    _ensure_axon_hooks()

    f32 = np.float32
    states = np.asarray(inputs["states"], f32)
    cls_token = np.asarray(inputs["cls_token"], f32)
    W1p = np.asarray(inputs["W1p"], f32)
    b1p = np.asarray(inputs["b1p"], f32)
    w2p = np.asarray(inputs["w2p"], f32)
    W1s = np.asarray(inputs["W1s"], f32)
    b1s = np.asarray(inputs["b1s"], f32)
    W2s = np.asarray(inputs["W2s"], f32)
    b2s = np.asarray(inputs["b2s"], f32)
    Wc1 = np.asarray(inputs["Wc1"], f32)
    bc1 = np.asarray(inputs["bc1"], f32)
    wc2 = np.asarray(inputs["wc2"], f32)
    bc2 = np.asarray(inputs["bc2"], f32)
    lengths = np.asarray(inputs["lengths"]).astype(np.int64)
    position_action = np.asarray(inputs["position_action"]).astype(np.int64)
    symbol_action = np.asarray(inputs["symbol_action"]).astype(np.int64)

    cores, NSL = _plan(lengths)
    W = NSL * RS
    AEND = (NSL - 1) * RS          # spkA data region size
    BS = (NSL - 2) * RS            # spkB covers packed [BS, W) + zero pad

    # static tail possible when every core's longest (last) row starts at
    # or before W-512, i.e. it covers the final 512 packed columns on its
    # own (the [Wc, W) remainder is zero-padding junk, masked out)
    ST = True
    for cs in cores:
        lns_c = [int(lengths[g]) for g in cs]
        if sum(lns_c) - lns_c[BC - 1] > W - 512:
            ST = False
            break

    # chunk tables per core.  Chunks are 128-col and row-aligned
    # ((j, L, src, cc)); in static mode the final 512 cols are instead
    # covered by 4 W-aligned chunks shared by all cores (dst rows 60-63).
    core_chunks = []
    NB = 0
    for c in range(NCORES):
        rows = cores[c]
        lns = [int(lengths[g]) for g in rows]
        offs = np.concatenate([[0], np.cumsum(lns)])[:BC]
        ch = []                    # (j, L, src, cc)
        for j, L in enumerate(lns):
            for cc in range((L + 127) // 128):
                srcv = int(offs[j]) + 128 * cc
                if ST and srcv >= W - 512:
                    break          # covered by the static tail chunks
                ch.append((j, L, srcv, cc))
        if ST:
            a, b = ch, []
        else:
            a = [t for t in ch if t[2] + 128 <= AEND]
            b = [t for t in ch if t[2] + 128 > AEND]
            NB = max(NB, len(b), 2)
        core_chunks.append((rows, lns, [int(x) for x in offs], a, b))
    NA0 = 60 if ST else 64 - NB
    for c in range(NCORES):
        rows, lns, offs, a, b = core_chunks[c]
        assert len(a) <= NA0, (len(a), NA0)
    key = (NSL, NB, ST)
    cfg = _cfg(NSL, NB, ST)

    # ---- shared (weight) tensors ----------------------------------------
    shared = {}
    # DoubleRow ct-major layout in 4 ct-pair chunks: [q, p, ct', ab, k2, jj, m]
    wq = (W1p * FP8_WSCALE).astype(ml_dtypes.float8_e4m3)
    wab = np.zeros((4, 128, 2, 2, K2, 2, 128), ml_dtypes.float8_e4m3)
    for ct in range(CT):
        for ab in range(2):
            half = wq[ab * E : (ab + 1) * E, ct * 128 : (ct + 1) * 128]
            for k2 in range(K2):
                for jj in range(2):
                    rws = half[256 * k2 + 128 * jj : 256 * k2 + 128 * (jj + 1)]
                    wab[ct // 2, :, ct % 2, ab, k2, jj, :] = rws
    shared["wab8"] = wab
    w2pm = np.zeros((128, 2, 16), np.float32)
    w2pm[:, :, : CT // 2] = w2p.reshape(CT // 2, 2, 128).transpose(2, 1, 0)
    shared["w2p8"] = _to_f8(w2pm * FP8_W2SCALE)

    aux32 = np.zeros((128, 2 * CT + KT + 1), f32)
    aux32[:, 0:CT] = b1p.reshape(CT, 128).T * FP8_WSCALE
    aux32[:, CT : 2 * CT] = b1s.reshape(CT, 128).T * FP8_WSCALE
    aux32[:, 2 * CT : 2 * CT + KT] = bc1.reshape(KT, 128).T
    aux32[0:BC, 2 * CT + KT] = bc2[0]
    shared["aux32"] = aux32

    ws8 = _to_f8((W1s * FP8_WSCALE).reshape(CT, 128, H).transpose(1, 0, 2))
    w2s8 = _to_f8((W2s * FP8_W2SCALE).reshape(CT, 128, A).transpose(1, 0, 2))
    shared["wsw2s8"] = np.concatenate(
        [ws8.reshape(128, CT * H), w2s8.reshape(128, CT * A)], axis=1
    )
    auxbf = np.zeros((128, KT + KT * BC + A + BC), f32)
    auxbf[:, 0:KT] = wc2.reshape(KT, 128).T
    auxbf[0, KT + KT * BC : KT + KT * BC + A] = b2s * FP8_WSCALE * FP8_W2SCALE
    auxbf[0, KT + KT * BC + A :] = 1.0
    shared["wc1"] = _to_cd(
        Wc1.reshape(KT, 128, E).transpose(1, 0, 2).reshape(128, KT * E)
    )

    # ---- per-core tensors ------------------------------------------------
    in_maps = []
    for c in range(NCORES):
        rows, lns, offs, a_ch, b_ch = core_chunks[c]

        # packed strip [E, W+1] (extra zero boundary col for the tail)
        xp = np.zeros((E, W + 1), ml_dtypes.float8_e4m3)
        for j, (g, L) in enumerate(zip(rows, lns)):
            xp[:, offs[j] : offs[j] + L] = states[g, :L].T.astype(
                ml_dtypes.float8_e4m3
            )
        xt8 = np.zeros((K2, NSL, 128, 2, XW), ml_dtypes.float8_e4m3)
        for k2 in range(K2):
            for s in range(NSL):
                for jj in range(2):
                    xt8[k2, s, :, jj, : RS + 1] = xp[
                        256 * k2 + 128 * jj : 256 * k2 + 128 * (jj + 1),
                        RS * s : RS * s + RS + 1,
                    ]

        # gather indices + mask/onehot/sel in chunk-row layout
        # pad rows point at offset 0: real, finite scores, fully masked
        NBPAD = NB
        gidxA = np.zeros((64, 1), np.int32)
        gidxB = np.zeros((max(NBPAD, 1), 1), np.int32)
        mask2 = np.full((64, 128), -1e30, f32)
        oh = np.zeros((72, 128), f32)
        sel = np.zeros((72, 2 * BC), f32)
        rowmap = {}
        for r, (j, L, srcv, cc) in enumerate(a_ch):
            gidxA[r, 0] = srcv
            rowmap[(j, cc)] = r
        for i, (j, L, srcv, cc) in enumerate(b_ch):
            r = 64 - NBPAD + i
            gidxB[i, 0] = srcv - BS
            rowmap[(j, cc)] = r
        for (j, cc), r in rowmap.items():
            L = lns[j]
            n = min(128, (L - 1) - 128 * cc)
            if ST:
                # elements at packed pos >= W-512 belong to the static rows
                n = min(n, (W - 512) - (offs[j] + 128 * cc))
            if n > 0:
                mask2[r, :n] = 0.0
            sel[r, j] = 1.0
        if ST:
            jl = BC - 1                    # the longest row (packed last)
            Ll = lns[jl]
            for q in range(4):
                r = 60 + q
                lo = W - 512 + 128 * q     # packed position of col 0
                n = min(128, (offs[jl] + Ll - 1) - lo)
                if n > 0:
                    mask2[r, max(0, offs[jl] - lo) : n] = 0.0
                sel[r, jl] = 1.0
        for j, g in enumerate(rows):
            pa = int(position_action[g])
            p = offs[j] + pa               # packed position of the action
            if ST and p >= W - 512:
                oh[60 + (p - (W - 512)) // 128, p % 128] = 1.0
            else:
                oh[rowmap[(j, pa // 128)], pa % 128] = 1.0
            oh[64 + j, int(symbol_action[g])] = 1.0
        for i in range(BC):
            sel[64 + i, BC + i] = 1.0
        aux32b = np.zeros((72, 128 + 2 * BC), f32)
        aux32b[:, 0:128] = oh
        aux32b[:, 128:] = sel

        pa_rows = position_action[rows]
        e12 = np.concatenate(
            [states[rows, pa_rows], states[rows, pa_rows + 1]], axis=1
        )                                      # (BC, 2E)
        abf = auxbf.copy()
        abf[:, KT : KT + KT * BC] = (
            cls_token[rows].T.reshape(KT, 128, BC).transpose(1, 0, 2)
            .reshape(128, KT * BC)
        )
        m = dict(shared)
        m["xt8"] = xt8
        m["gidxA"] = gidxA
        if not ST:
            m["gidxB"] = gidxB
        m["mask2"] = mask2
        m["aux32b"] = aux32b
        m["auxbf"] = _to_cd(abf)
        m["e12t"] = _to_f8(
            e12.T.reshape(CT, 128, BC).transpose(1, 0, 2).reshape(128, CT * BC)
        )
        in_maps.append(m)

    if key not in _CACHED:
        _CACHED[key] = _build(cfg)
    nc = _CACHED[key]

    # cold first execution of a freshly-loaded NEFF measures ~15-20% slow
    # (device-side warmup); run once untimed, then the traced run
    run_bass_kernel_spmd(nc, in_maps, core_ids=list(range(NCORES)), trace=False)
    try:
        res = run_bass_kernel_spmd(
            nc, in_maps, core_ids=list(range(NCORES)), trace=TRACE
        )
    except (ImportError, ModuleNotFoundError):
        res = run_bass_kernel_spmd(
            nc, in_maps, core_ids=list(range(NCORES)), trace=False
        )
    LAST_EXEC_NS = res.exec_time_ns

    full = np.zeros((B, 5), f32)
    for c in range(NCORES):
        o = np.asarray(res.results[c]["out"])
        for j, g in enumerate(cores[c]):
            full[g] = o[j]
    return np.ascontiguousarray(full.T, dtype=f32)  # (5, 64)


# revision 38
# speedup vs baseline: 1.0083x; 1.0083x over previous
"""Trainium2 Bass kernel for the ActorCritic ragged-sequence problem.

Strategy (v8: tight ragged packing + static tail)
-------------------------------------------------
Data-parallel over batch B=64 across 8 NeuronCores, computing pair scores
only for each row's valid prefix (lengths are ragged in [2, S=1024]).
Rows are assigned to cores by cardinality-constrained LPT (+local swaps),
each core packs its 8 rows back-to-back into a W = NSL*512 column strip
(NSL = 9 for the reference lengths vs 16 dense: ~1.8x less matmul work),
with its longest row packed last so the final 512 columns belong to one
row on every core.

Per core the pair-MLP h = relu(x_t @ W1a + x_{t+1} @ W1b + b1p),
score = w2p.h runs as weight-stationary fp8 DoubleRow matmuls (K=256 per
instruction, FD=512) over the packed strip; the +1 shift of the pair's
second element is a one-element slice offset into each 528-wide window.
Scores land in a packed [1, W] SBUF row; slices 0..NSL-2 stream to a
small Internal-DRAM scratch, from which ONE indirect element-granular
gather (SWDGE, per-core chunk-offset table as data) pulls per-row-aligned
128-col chunks onto partitions 0..59 - it fires under the last slice's
compute.  The final 512 columns bypass DRAM entirely via a static
SBUF->SBUF DMA into partitions 60-63.  A [72, 128] exp/entropy pass
(rows 64-71 hold the symbol-head logits, moved there by a tiny DMA)
produces per-partition partials which two small matmuls against 0/1
selection matrices (per-core data) combine into per-row Z/S2/logp;
masks at -1e30 kill row-boundary junk and padding.  If a pathological
length set breaks the static-tail precondition, a second indirect gather
covers the last slices instead (both variants compiled on demand, cached
by (NSL, NB, static)).

The symbol head runs in fp8 (weights prescaled by powers of two, undone
exactly on chip), the critic in bf16; both are emitted mid-loop so their
matmuls hide in the PE stream.  DMA queues: sync HWDGE carries the
x-window stream + main weights (merged transfers, issue order = arrival
order via order-only dep edges); gpsimd SWDGE carries aux/late weights so
the scalar engine stays free for activations.  76 dummy warm-up matmuls
lift the PE HAM clock gate during the initial DMA fill.  Host work is
pure indexing / layout / quantization - no FLOPs moved off-device.

Measured on trn2 (8 cores): ~97-102 us HW exec (baseline 185 us), rel
err ~2e-3 vs the fp32 reference (gate 2e-2).
"""

import os
import numpy as np

B, S, E, A = 64, 1024, 512, 128
NCORES = 8
BC = B // NCORES          # batch rows per core (= slots per core)
H = 2 * E                 # pair-MLP hidden dim
RS = 512                  # matmul moving free dim per slice
KT = E // 128             # 4 k-tiles over the E features
K2 = KT // 2              # 2 fp8 DoubleRow k-tiles (K=256 each)
CT = H // 128             # 8 chan tiles of the hidden dim
XW = 512 + 16             # padded window width (512 cols + boundary + pad)
VCT = E // 128            # chan tiles of the critic hidden dim

TRACE = os.environ.get("K_TRACE", "1") == "1"

LAST_EXEC_NS = None
_CACHED = {}

_LDWOPT = os.environ.get("K_LDWOPT", "0") == "1"
_PATCHED = False

FP8_WSCALE = 32.0    # power-of-two prescale keeping fp8 W1p/W1s mid-range
FP8_W2SCALE = 256.0  # prescale for w2p/W2s; undone exactly on chip


def _patch_walrus_flags():
    """Re-enable walrus LDWEIGHTS dedup (repeated stationary operands) for
    this process's compiles."""
    global _PATCHED
    if _PATCHED or not _LDWOPT:
        return
    import concourse.bass_utils as _bu

    _orig = _bu.run_command

    def _rc(argv, **kw):
        argv = [
            "--enable-ldw-opt=true" if a == "--enable-ldw-opt=false" else a
            for a in argv
        ]
        return _orig(argv, **kw)

    _bu.run_command = _rc
    _PATCHED = True


def _plan(lengths):
    """LPT assignment of the 64 rows to 8 cores (tight packing): returns
    (cores: list of 8 row-lists, NSL)."""
    ln = np.asarray(lengths).astype(np.int64)
    order = np.argsort(-ln, kind="stable")
    cores = [[] for _ in range(NCORES)]
    sums = np.zeros(NCORES, np.int64)
    for g in order:
        open_c = [c for c in range(NCORES) if len(cores[c]) < BC]
        c = min(open_c, key=lambda c: sums[c])
        cores[c].append(int(g))
        sums[c] += ln[g]
    # local improvement: swap rows between the fullest core and others
    for _ in range(200):
        hi = int(np.argmax(sums))
        best = None
        for c in range(NCORES):
            if c == hi:
                continue
            for i, gi in enumerate(cores[hi]):
                for k, gk in enumerate(cores[c]):
                    delta = int(ln[gi] - ln[gk])
                    if delta <= 0:
                        continue
                    new_hi = sums[hi] - delta
                    new_c = sums[c] + delta
                    new_max = max(new_hi, new_c)
                    if new_max < sums[hi] and (best is None or new_max < best[0]):
                        best = (new_max, c, i, k, delta)
        if best is None:
            break
        _, c, i, k, delta = best
        cores[hi][i], cores[c][k] = cores[c][k], cores[hi][i]
        sums[hi] -= delta
        sums[c] += delta
    NSL = (int(sums.max()) + RS - 1) // RS
    # put each core's longest row last (enables the static tail chunks)
    out = []
    for cs in cores:
        cs = list(map(int, cs))
        jmax = max(range(BC), key=lambda j: ln[cs[j]])
        cs.append(cs.pop(jmax))
        out.append(cs)
    return out, NSL


def _cfg(NSL, NB, ST):
    return dict(NSL=NSL, W=NSL * RS, NB=NB, ST=ST)


def _build(cfg):
    import concourse.tile as tile
    from concourse import bacc, mybir
    from concourse.tile_rust import add_dep_helper

    _patch_walrus_flags()

    F32 = mybir.dt.float32
    BF16 = mybir.dt.bfloat16
    F8 = mybir.dt.float8e4
    AF = mybir.ActivationFunctionType
    OP = mybir.AluOpType
    AX = mybir.AxisListType
    DR = mybir.MatmulPerfMode.DoubleRow

    NSL, W, NB, ST = cfg["NSL"], cfg["W"], cfg["NB"], cfg["ST"]
    NA0 = 60 if ST else 64 - NB
    NBPAD = NB
    BS = (NSL - 2) * RS   # spkB DRAM scratch covers packed cols [BS, W+128)

    nc = bacc.Bacc("TRN2", target_bir_lowering=False, debug=False)

    # ---- DRAM parameters -------------------------------------------------
    # packed, pair-interleaved fp8 states: [k2, window, part, plane, col]
    xt_d = nc.dram_tensor("xt8", [K2, NSL, 128, 2, XW], F8, kind="ExternalInput")
    # ct-major pair-MLP weights in 4 ct-pair chunks: [q, p, ct', ab, k2, jj, m]
    wab_d = nc.dram_tensor("wab8", [4, 128, 2, 2, K2, 2, 128], F8,
                           kind="ExternalInput")
    w2p_d = nc.dram_tensor("w2p8", [128, 2, 16], F8, kind="ExternalInput")
    mask_d = nc.dram_tensor("mask2", [64, 128], F32, kind="ExternalInput")
    idxA_d = nc.dram_tensor("gidxA", [64, 1], mybir.dt.int32, kind="ExternalInput")
    idxB_d = None
    if not ST:
        idxB_d = nc.dram_tensor("gidxB", [NBPAD, 1], mybir.dt.int32,
                                kind="ExternalInput")
    spkA_d = nc.dram_tensor("spkA", [(NSL - 1) * RS + (128 if ST else 0), 1],
                            F32, kind="Internal")
    spkB_d = None
    if not ST:
        spkB_d = nc.dram_tensor("spkB", [2 * RS + 128, 1], F32,
                                kind="Internal")
    # merged aux: fp32 [b1p | b1s | bc1 | bc2]
    aux32_d = nc.dram_tensor("aux32", [128, 2 * CT + KT + 1], F32,
                             kind="ExternalInput")
    # merged aux: fp32 [oh_all | sel]
    aux32b_d = nc.dram_tensor("aux32b", [72, 128 + 2 * BC], F32,
                              kind="ExternalInput")
    # merged aux: bf16 [wc2 | clst | b2s,ones on partition 0]
    auxbf_d = nc.dram_tensor("auxbf", [128, KT + KT * BC + A + BC], BF16,
                             kind="ExternalInput")
    e12_d = nc.dram_tensor("e12t", [128, CT * BC], F8, kind="ExternalInput")
    wsw2s_d = nc.dram_tensor("wsw2s8", [128, CT * H + CT * A], F8,
                             kind="ExternalInput")
    wc1_d = nc.dram_tensor("wc1", [128, KT * E], BF16, kind="ExternalInput")
    out_d = nc.dram_tensor("out", [BC, 5], F32, kind="ExternalOutput")

    with tile.TileContext(nc) as tc:
        with (
            tc.tile_pool(name="weights", bufs=1) as wpool,
            tc.tile_pool(name="hbuf", bufs=2) as hpool,
            tc.tile_pool(name="small", bufs=1) as spool,
            tc.tile_pool(name="psmain", bufs=2, space="PSUM") as psmain,
            tc.tile_pool(name="pssc", bufs=2, space="PSUM") as pssc,
            tc.tile_pool(name="ps3", bufs=2, space="PSUM") as ps3,
        ):
            # ---- PE warm-up: dummy matmuls during the initial DMA fill ---
            wtmp = spool.tile([128, 64], F8, name="wtmp")
            nc.vector.memset(wtmp[:], 0.0)
            for i in range(76):
                pw = psmain.tile([64, 64], F32, name="pw", tag=f"ps{i % 2}")
                nc.tensor.matmul(pw[:], wtmp[:], wtmp[:], start=True, stop=True)

            # ---- sync HWDGE queue: x windows + main weights --------------
            xbf = {}
            sync_dmas = []

            def qsync(dst, src):
                dma = nc.sync.dma_start(dst, src)
                if sync_dmas:
                    add_dep_helper(dma.ins, sync_dmas[-1].ins, False,
                                   "sync dma issue order")
                sync_dmas.append(dma)
                return dma

            wab_sb = [wpool.tile([128, 2, 2, K2, 2, 128], F8,
                                 name=f"wabq{q}") for q in range(4)]

            def xwin(s):
                for k2 in range(K2):
                    t = wpool.tile([128, 2, XW], F8, name=f"x8_{k2}_{s}")
                    xbf[(k2, s)] = t
                    qsync(t[:], xt_d[k2, s, :, :, :])

            # x windows stream alone on the sync queue; the weight chunks
            # arrive in parallel on the gpsimd queue
            for s in range(NSL):
                xwin(s)

            def wab_ap(ct, ab, k2):
                return wab_sb[ct // 2][:, ct % 2, ab, k2, :, :]

            # ---- gpsimd SWDGE queue: aux + symbol/critic weights ---------
            gp_dmas = []

            def qgp(dst, src):
                dma = nc.gpsimd.dma_start(dst, src)
                if gp_dmas:
                    add_dep_helper(dma.ins, gp_dmas[-1].ins, False,
                                   "gpsimd dma issue order")
                gp_dmas.append(dma)
                return dma

            aux32_sb = wpool.tile([128, 2 * CT + KT + 1], F32, name="aux32")
            qgp(aux32_sb[:], aux32_d[:, :])
            for q in range(4):
                qgp(wab_sb[q][:], wab_d[q, :, :, :, :, :, :])
            w2p_sb = wpool.tile([128, 2, 16], F8, name="w2p")
            qgp(w2p_sb[:], w2p_d[:, :, :])
            sm_all = spool.tile([72, 128], F32, name="small")
            mask_sb = wpool.tile([64, 128], F32, name="mask2")
            qgp(mask_sb[:], mask_d[:, :])
            idxA_sb = wpool.tile([64, 1], mybir.dt.int32, name="gidxA")
            qgp(idxA_sb[:], idxA_d[:, :])
            if not ST:
                idxB_sb = wpool.tile([NBPAD, 1], mybir.dt.int32, name="gidxB")
                qgp(idxB_sb[:], idxB_d[:, :])
            e12_sb = wpool.tile([128, CT * BC], F8, name="e12")
            qgp(e12_sb[:], e12_d[:, :])
            aux32b_sb = wpool.tile([72, 128 + 2 * BC], F32, name="aux32b")
            qgp(aux32b_sb[:], aux32b_d[:, :])
            auxbf_sb = wpool.tile([128, KT + KT * BC + A + BC], BF16,
                                  name="auxbf")
            qgp(auxbf_sb[:], auxbf_d[:, :])
            wsw2s_sb = wpool.tile([128, CT * H + CT * A], F8, name="wsw2s")
            qgp(wsw2s_sb[:], wsw2s_d[:, :])
            wc1_sb = wpool.tile([128, KT * E], BF16, name="wc1")
            qgp(wc1_sb[:], wc1_d[:, :])

            # ---- packed score row + tail tiles ---------------------------
            scores_pk = spool.tile([1, W], F32, name="scpk")
            zt = spool.tile([1, 128], F32, name="zt")
            nc.vector.memset(zt[:], 0.0)
            if ST:
                nc.sync.dma_start(
                    spkA_d[(NSL - 1) * RS : (NSL - 1) * RS + 128, 0:1], zt[:]
                )
            else:
                nc.sync.dma_start(spkB_d[2 * RS : 2 * RS + 128, 0:1], zt[:])
            smy_tmp = spool.tile([BC, A], F32, name="smyt")
            outbuf = spool.tile([BC, 5], F32, name="outbuf")

            # preload the Exp activation table off the critical path
            dume = spool.tile([1, 16], F32, name="dume")
            nc.scalar.activation(dume[:], wtmp[0:1, 0:16], AF.Exp)

            def emit_symcrit():
                # symbol head (fp8, scaled by 32/256, undone on copy)
                sh_sb = [spool.tile([128, BC], F8, name=f"sh{ct}")
                         for ct in range(CT)]
                for ct in range(CT):
                    p3 = ps3.tile([128, BC], F32, name="p3", tag="p3")
                    for k in range(CT):
                        nc.tensor.matmul(
                            p3[:],
                            wsw2s_sb[:, k * H + ct * 128 : k * H + (ct + 1) * 128],
                            e12_sb[:, k * BC : (k + 1) * BC],
                            start=(k == 0),
                            stop=(k == CT - 1),
                        )
                    nc.scalar.activation(
                        sh_sb[ct][:], p3[:], AF.Relu,
                        bias=aux32_sb[:, CT + ct : CT + ct + 1],
                    )
                psl = ps3.tile([BC, A], F32, name="psl", tag="p3")
                for ct in range(CT):
                    nc.tensor.matmul(
                        psl[:], sh_sb[ct][:],
                        wsw2s_sb[:, CT * H + ct * A : CT * H + (ct + 1) * A],
                        start=(ct == 0), stop=False,
                    )
                nc.tensor.matmul(
                    psl[:],
                    auxbf_sb[0:1, KT + KT * BC + A : KT + KT * BC + A + BC],
                    auxbf_sb[0:1, KT + KT * BC : KT + KT * BC + A],
                    start=False, stop=True,
                )
                # rescale at partitions 0-7, then DMA to partitions 64-71
                # (engine ops are partition-locked, DMAs are not)
                nc.scalar.activation(
                    smy_tmp[:], psl[:], AF.Copy, bias=0.0, scale=1.0 / 8192.0
                )
                nc.sync.dma_start(sm_all[64:72, :], smy_tmp[:])

                # critic (bf16)
                hc_sb = [spool.tile([128, BC], BF16, name=f"hc{ct}")
                         for ct in range(VCT)]
                for ct in range(VCT):
                    pc = ps3.tile([128, BC], F32, name="pc", tag="p3")
                    for k in range(KT):
                        nc.tensor.matmul(
                            pc[:],
                            wc1_sb[:, k * E + ct * 128 : k * E + (ct + 1) * 128],
                            auxbf_sb[:, KT + k * BC : KT + (k + 1) * BC],
                            start=(k == 0),
                            stop=(k == KT - 1),
                        )
                    nc.scalar.activation(
                        hc_sb[ct][:], pc[:], AF.Relu,
                        bias=aux32_sb[:, 2 * CT + ct : 2 * CT + ct + 1],
                    )
                pv = ps3.tile([BC, 1], F32, name="pv", tag="p3")
                for ct in range(VCT):
                    nc.tensor.matmul(
                        pv[:], hc_sb[ct][:], auxbf_sb[:, ct : ct + 1],
                        start=(ct == 0), stop=(ct == VCT - 1),
                    )
                nc.vector.tensor_add(outbuf[:, 2:3], pv[:],
                                     aux32_sb[0:BC, 2 * CT + KT : 2 * CT + KT + 1])  # val

            SYM_AT = min(2, NSL - 1)

            # ---- main pair-MLP over packed slices ------------------------
            for s in range(NSL):
                FD = RS
                hs = {}
                for ct in range(CT):
                    ps = psmain.tile([128, RS], F32, name=f"ps{s}_{ct}",
                                     tag=f"ps{s % 2}")
                    for wi in range(4):
                        ab, k2 = divmod(wi, K2)
                        nc.tensor.matmul(
                            ps[:, :FD],
                            wab_ap(ct, ab, k2),
                            xbf[(k2, s)][:, :, ab : ab + FD],
                            start=(wi == 0),
                            stop=(wi == 3),
                            perf_mode=DR,
                        )
                    m, jj = divmod(ct, 2)
                    if (s, m) not in hs:
                        hs[(s, m)] = hpool.tile([128, 2, RS], F8,
                                                name=f"h8_{m}", tag=f"h8_{m}")
                    plane = hs[(s, m)][:, jj, :FD]
                    # split bias+relu ~2:1 DVE:ACT so both stay in PE shadow
                    if (s * CT + ct) % 3 == 2:
                        nc.scalar.activation(
                            plane, ps[:, :FD], AF.Relu,
                            bias=aux32_sb[:, ct : ct + 1],
                        )
                    else:
                        nc.vector.tensor_scalar(
                            plane, ps[:, :FD], aux32_sb[:, ct : ct + 1], 0.0,
                            OP.add, OP.max,
                        )
                psd = pssc.tile([1, RS], F32, name="psd", tag="psd")
                for m in range(CT // 2):
                    nc.tensor.matmul(
                        psd[:, :FD],
                        w2p_sb[:, :, m : m + 1],
                        hs[(s, m)][:, :, :FD],
                        start=(m == 0),
                        stop=(m == CT // 2 - 1),
                        perf_mode=DR,
                    )
                nc.scalar.activation(
                    scores_pk[0:1, s * RS : s * RS + FD], psd[:, :FD],
                    AF.Copy, bias=0.0, scale=1.0 / 8192.0,
                )
                strip = scores_pk[0:1, s * RS : s * RS + FD]
                if s <= NSL - 2:
                    nc.sync.dma_start(
                        spkA_d[s * RS : s * RS + FD, 0:1], strip
                    )
                if not ST and s >= NSL - 2:
                    nc.sync.dma_start(
                        spkB_d[s * RS - BS : s * RS - BS + FD, 0:1], strip
                    )
                if s == SYM_AT:
                    # interleave the (tiny) symbol head + critic here: their
                    # weights have landed by now and the PE queue is in-order
                    emit_symcrit()

            # ---- accumulate-scatter packed scores onto the mask-prefilled
            # [64, 128] chunk layout (slot j -> partitions 8j..8j+chunks) --
            # two indirect element-granular gathers pull the per-row-aligned
            # chunks from the DRAM score scratch: rows [0:NA0] depend only on
            # slices <= NSL-2 (spkA), rows [NA0:64] on the final slices
            # (spkB).  Chunk indices are per-core DATA (tight LPT packing).
            from concourse.bass import IndirectOffsetOnAxis

            scr2 = spool.tile([64, 128], F32, name="scr2")
            nc.gpsimd.indirect_dma_start(
                scr2[0:NA0, :], None,
                spkA_d[:, :],
                IndirectOffsetOnAxis(ap=idxA_sb[0:NA0, 0:1], axis=0),
            )
            if ST:
                # last 512 packed cols are the longest row's tail on every
                # core: a single static SBUF->SBUF chunk DMA, no DRAM hop
                nc.sync.dma_start(
                    scr2[60:64, :], scores_pk[0:1, W - 512 : W]
                )
            else:
                nc.gpsimd.indirect_dma_start(
                    scr2[64 - NBPAD : 64, :], None,
                    spkB_d[:, :],
                    IndirectOffsetOnAxis(ap=idxB_sb[:, :], axis=0),
                )
            nc.vector.tensor_add(sm_all[0:64, :], scr2[:], mask_sb[:])

            # ---- softmax statistics over [72, 128] -----------------------
            pexp = spool.tile([72, 128], F32, name="pexp")
            pcols = spool.tile([72, 3], F32, name="pcols")
            tmp = spool.tile([72, 128], F32, name="tmpa")
            p2 = spool.tile([72, 128], F32, name="p2")
            nc.scalar.activation(
                pexp[:], sm_all[:], AF.Exp, accum_out=pcols[:, 0:1]
            )
            nc.vector.tensor_mul(tmp[:], sm_all[:], aux32b_sb[:, 0:128])
            nc.vector.tensor_reduce(pcols[:, 2:3], tmp[:], axis=AX.X, op=OP.add)
            nc.vector.tensor_mul(p2[:], pexp[:], sm_all[:])
            nc.vector.tensor_reduce(pcols[:, 1:2], p2[:], axis=AX.X, op=OP.add)

            # ---- per-row combine via tiny matmuls (psB's operands both
            # live at base partition 64 so the contraction indices align) --
            psA = ps3.tile([BC, 3], F32, name="psA", tag="p3")
            nc.tensor.matmul(psA[:], aux32b_sb[0:64, 128 : 128 + BC], pcols[0:64, :],
                             start=True, stop=True)
            psB = ps3.tile([BC, 3], F32, name="psB", tag="p3")
            nc.tensor.matmul(psB[:], aux32b_sb[64:72, 128 + BC : 128 + 2 * BC],
                             pcols[64:72, :], start=True, stop=True)

            lseA = spool.tile([BC, 1], F32, name="lseA")
            lseB = spool.tile([BC, 1], F32, name="lseB")
            nc.scalar.activation(lseA[:], psA[:, 0:1], AF.Ln)
            nc.scalar.activation(lseB[:], psB[:, 0:1], AF.Ln)
            rzA = spool.tile([BC, 1], F32, name="rzA")
            rzB = spool.tile([BC, 1], F32, name="rzB")
            nc.vector.reciprocal(rzA[:], psA[:, 0:1])
            nc.vector.reciprocal(rzB[:], psB[:, 0:1])
            s2zA = spool.tile([BC, 1], F32, name="s2zA")
            s2zB = spool.tile([BC, 1], F32, name="s2zB")
            nc.vector.tensor_mul(s2zA[:], psA[:, 1:2], rzA[:])
            nc.vector.tensor_mul(s2zB[:], psB[:, 1:2], rzB[:])
            nc.vector.tensor_sub(outbuf[:, 0:1], psA[:, 2:3], lseA[:])  # logp_pos
            nc.vector.tensor_sub(outbuf[:, 1:2], psB[:, 2:3], lseB[:])  # logp_sym
            nc.vector.tensor_sub(outbuf[:, 3:4], lseA[:], s2zA[:])      # ent_pos
            nc.vector.tensor_sub(outbuf[:, 4:5], lseB[:], s2zB[:])      # ent_sym

            nc.sync.dma_start(out_d[:, :], outbuf[:])

    nc.compile()
    return nc


def _to_cd(arr):
    import ml_dtypes

    return np.ascontiguousarray(arr).astype(ml_dtypes.bfloat16)


def _to_f8(arr):
    import ml_dtypes

    return np.ascontiguousarray(arr).astype(ml_dtypes.float8_e4m3)


def _ntff_profile_via_ctypes(so_path):
    """(dir, device_ids) -> contextmanager hook driving NTFF profiling via
    ctypes calls into the axon PJRT .so (mirrors the boot-side helper)."""
    import contextlib
    import ctypes
    import sys

    try:
        lib = ctypes.CDLL(so_path)
    except OSError:
        return None
    if not hasattr(lib, "axon_start_nrt_profile"):
        return None
    lib.axon_start_nrt_profile.argtypes = [
        ctypes.POINTER(ctypes.c_int64),
        ctypes.c_size_t,
    ]
    lib.axon_start_nrt_profile.restype = ctypes.c_int64
    lib.axon_stop_nrt_profile.argtypes = [ctypes.c_char_p]
    lib.axon_stop_nrt_profile.restype = ctypes.c_int64

    @contextlib.contextmanager
    def _hook(output_dir, device_ids):
        import jax

        jax.devices()
        if device_ids:
            ids = (ctypes.c_int64 * len(device_ids))(*device_ids)
            rc = lib.axon_start_nrt_profile(ids, len(device_ids))
        else:
            rc = lib.axon_start_nrt_profile(None, 0)
        if rc != 0:
            raise RuntimeError(f"axon_start_nrt_profile rc={rc}")
        try:
            yield
        finally:
            n = lib.axon_stop_nrt_profile(str(output_dir).encode())
            if n < 0:
                raise RuntimeError(f"axon_stop_nrt_profile rc={n}")
            print(f"profile: {n} file(s) written to {output_dir}", file=sys.stderr)

    return _hook


def _ensure_axon_hooks():
    """bass_utils imports antenv.axon_hooks unconditionally when tracing
    under axon; provide a registry (with the real ctypes-backed NTFF hook
    when the axon .so is present) if the image lacks it."""
    try:
        import antenv.axon_hooks as _h  # noqa: F401
        if _h.get_axon_ntff_profile_hook() is None:
            hook = _ntff_profile_via_ctypes("/opt/axon/libaxon_pjrt.so")
            if hook is not None:
                _h.set_axon_ntff_profile_hook(hook)
        return
    except ImportError:
        pass
    import sys
    import types

    try:
        import antenv
    except ImportError:
        return
    mod = types.ModuleType("antenv.axon_hooks")
    mod._hook = _ntff_profile_via_ctypes("/opt/axon/libaxon_pjrt.so")
    mod.set_axon_ntff_profile_hook = lambda h: setattr(mod, "_hook", h)
    mod.get_axon_ntff_profile_hook = lambda: mod._hook
    sys.modules["antenv.axon_hooks"] = mod
    antenv.axon_hooks = mod


def kernel(**inputs):
    global LAST_EXEC_NS
    import ml_dtypes
    from concourse.bass_utils import run_bass_kernel_spmd

    _ensure_axon_hooks()

    f32 = np.float32
    states = np.asarray(inputs["states"], f32)
    cls_token = np.asarray(inputs["cls_token"], f32)
    W1p = np.asarray(inputs["W1p"], f32)
    b1p = np.asarray(inputs["b1p"], f32)
    w2p = np.asarray(inputs["w2p"], f32)
    W1s = np.asarray(inputs["W1s"], f32)
    b1s = np.asarray(inputs["b1s"], f32)
    W2s = np.asarray(inputs["W2s"], f32)
    b2s = np.asarray(inputs["b2s"], f32)
    Wc1 = np.asarray(inputs["Wc1"], f32)
    bc1 = np.asarray(inputs["bc1"], f32)
    wc2 = np.asarray(inputs["wc2"], f32)
    bc2 = np.asarray(inputs["bc2"], f32)
    lengths = np.asarray(inputs["lengths"]).astype(np.int64)
    position_action = np.asarray(inputs["position_action"]).astype(np.int64)
    symbol_action = np.asarray(inputs["symbol_action"]).astype(np.int64)

    cores, NSL = _plan(lengths)
    W = NSL * RS
    AEND = (NSL - 1) * RS          # spkA data region size
    BS = (NSL - 2) * RS            # spkB covers packed [BS, W) + zero pad

    # static tail possible when every core's longest (last) row starts at
    # or before W-512, i.e. it covers the final 512 packed columns on its
    # own (the [Wc, W) remainder is zero-padding junk, masked out)
    ST = True
    for cs in cores:
        lns_c = [int(lengths[g]) for g in cs]
        if sum(lns_c) - lns_c[BC - 1] > W - 512:
            ST = False
            break

    # chunk tables per core.  Chunks are 128-col and row-aligned
    # ((j, L, src, cc)); in static mode the final 512 cols are instead
    # covered by 4 W-aligned chunks shared by all cores (dst rows 60-63).
    core_chunks = []
    NB = 0
    for c in range(NCORES):
        rows = cores[c]
        lns = [int(lengths[g]) for g in rows]
        offs = np.concatenate([[0], np.cumsum(lns)])[:BC]
        ch = []                    # (j, L, src, cc)
        for j, L in enumerate(lns):
            for cc in range((L + 127) // 128):
                srcv = int(offs[j]) + 128 * cc
                if ST and srcv >= W - 512:
                    break          # covered by the static tail chunks
                ch.append((j, L, srcv, cc))
        if ST:
            a, b = ch, []
        else:
            a = [t for t in ch if t[2] + 128 <= AEND]
            b = [t for t in ch if t[2] + 128 > AEND]
            NB = max(NB, len(b), 2)
        core_chunks.append((rows, lns, [int(x) for x in offs], a, b))
    NA0 = 60 if ST else 64 - NB
    for c in range(NCORES):
        rows, lns, offs, a, b = core_chunks[c]
        assert len(a) <= NA0, (len(a), NA0)
    key = (NSL, NB, ST)
    cfg = _cfg(NSL, NB, ST)

    # ---- shared (weight) tensors ----------------------------------------
    shared = {}
    # DoubleRow ct-major layout in 4 ct-pair chunks: [q, p, ct', ab, k2, jj, m]
    wq = (W1p * FP8_WSCALE).astype(ml_dtypes.float8_e4m3)
    wab = np.zeros((4, 128, 2, 2, K2, 2, 128), ml_dtypes.float8_e4m3)
    for ct in range(CT):
        for ab in range(2):
            half = wq[ab * E : (ab + 1) * E, ct * 128 : (ct + 1) * 128]
            for k2 in range(K2):
                for jj in range(2):
                    rws = half[256 * k2 + 128 * jj : 256 * k2 + 128 * (jj + 1)]
                    wab[ct // 2, :, ct % 2, ab, k2, jj, :] = rws
    shared["wab8"] = wab
    w2pm = np.zeros((128, 2, 16), np.float32)
    w2pm[:, :, : CT // 2] = w2p.reshape(CT // 2, 2, 128).transpose(2, 1, 0)
    shared["w2p8"] = _to_f8(w2pm * FP8_W2SCALE)

    aux32 = np.zeros((128, 2 * CT + KT + 1), f32)
    aux32[:, 0:CT] = b1p.reshape(CT, 128).T * FP8_WSCALE
    aux32[:, CT : 2 * CT] = b1s.reshape(CT, 128).T * FP8_WSCALE
    aux32[:, 2 * CT : 2 * CT + KT] = bc1.reshape(KT, 128).T
    aux32[0:BC, 2 * CT + KT] = bc2[0]
    shared["aux32"] = aux32

    ws8 = _to_f8((W1s * FP8_WSCALE).reshape(CT, 128, H).transpose(1, 0, 2))
    w2s8 = _to_f8((W2s * FP8_W2SCALE).reshape(CT, 128, A).transpose(1, 0, 2))
    shared["wsw2s8"] = np.concatenate(
        [ws8.reshape(128, CT * H), w2s8.reshape(128, CT * A)], axis=1
    )
    auxbf = np.zeros((128, KT + KT * BC + A + BC), f32)
    auxbf[:, 0:KT] = wc2.reshape(KT, 128).T
    auxbf[0, KT + KT * BC : KT + KT * BC + A] = b2s * FP8_WSCALE * FP8_W2SCALE
    auxbf[0, KT + KT * BC + A :] = 1.0
    shared["wc1"] = _to_cd(
        Wc1.reshape(KT, 128, E).transpose(1, 0, 2).reshape(128, KT * E)
    )

    # ---- per-core tensors ------------------------------------------------
    in_maps = []
    for c in range(NCORES):
        rows, lns, offs, a_ch, b_ch = core_chunks[c]

        # packed strip [E, W+1] (extra zero boundary col for the tail)
        xp = np.zeros((E, W + 1), ml_dtypes.float8_e4m3)
        for j, (g, L) in enumerate(zip(rows, lns)):
            xp[:, offs[j] : offs[j] + L] = states[g, :L].T.astype(
                ml_dtypes.float8_e4m3
            )
        xt8 = np.zeros((K2, NSL, 128, 2, XW), ml_dtypes.float8_e4m3)
        for k2 in range(K2):
            for s in range(NSL):
                for jj in range(2):
                    xt8[k2, s, :, jj, : RS + 1] = xp[
                        256 * k2 + 128 * jj : 256 * k2 + 128 * (jj + 1),
                        RS * s : RS * s + RS + 1,
                    ]

        # gather indices + mask/onehot/sel in chunk-row layout
        # pad rows point at offset 0: real, finite scores, fully masked
        NBPAD = NB
        gidxA = np.zeros((64, 1), np.int32)
        gidxB = np.zeros((max(NBPAD, 1), 1), np.int32)
        mask2 = np.full((64, 128), -1e30, f32)
        oh = np.zeros((72, 128), f32)
        sel = np.zeros((72, 2 * BC), f32)
        rowmap = {}
        for r, (j, L, srcv, cc) in enumerate(a_ch):
            gidxA[r, 0] = srcv
            rowmap[(j, cc)] = r
        for i, (j, L, srcv, cc) in enumerate(b_ch):
            r = 64 - NBPAD + i
            gidxB[i, 0] = srcv - BS
            rowmap[(j, cc)] = r
        for (j, cc), r in rowmap.items():
            L = lns[j]
            n = min(128, (L - 1) - 128 * cc)
            if ST:
                # elements at packed pos >= W-512 belong to the static rows
                n = min(n, (W - 512) - (offs[j] + 128 * cc))
            if n > 0:
                mask2[r, :n] = 0.0
            sel[r, j] = 1.0
        if ST:
            jl = BC - 1                    # the longest row (packed last)
            Ll = lns[jl]
            for q in range(4):
                r = 60 + q
                lo = W - 512 + 128 * q     # packed position of col 0
                n = min(128, (offs[jl] + Ll - 1) - lo)
                if n > 0:
                    mask2[r, max(0, offs[jl] - lo) : n] = 0.0
                sel[r, jl] = 1.0
        for j, g in enumerate(rows):
            pa = int(position_action[g])
            p = offs[j] + pa               # packed position of the action
            if ST and p >= W - 512:
                oh[60 + (p - (W - 512)) // 128, p % 128] = 1.0
            else:
                oh[rowmap[(j, pa // 128)], pa % 128] = 1.0
            oh[64 + j, int(symbol_action[g])] = 1.0
        for i in range(BC):
            sel[64 + i, BC + i] = 1.0
        aux32b = np.zeros((72, 128 + 2 * BC), f32)
        aux32b[:, 0:128] = oh
        aux32b[:, 128:] = sel

        pa_rows = position_action[rows]
        e12 = np.concatenate(
            [states[rows, pa_rows], states[rows, pa_rows + 1]], axis=1
        )                                      # (BC, 2E)
        abf = auxbf.copy()
        abf[:, KT : KT + KT * BC] = (
            cls_token[rows].T.reshape(KT, 128, BC).transpose(1, 0, 2)
            .reshape(128, KT * BC)
        )
        m = dict(shared)
        m["xt8"] = xt8
        m["gidxA"] = gidxA
        if not ST:
            m["gidxB"] = gidxB
        m["mask2"] = mask2
        m["aux32b"] = aux32b
        m["auxbf"] = _to_cd(abf)
        m["e12t"] = _to_f8(
            e12.T.reshape(CT, 128, BC).transpose(1, 0, 2).reshape(128, CT * BC)
        )
        in_maps.append(m)

    if key not in _CACHED:
        _CACHED[key] = _build(cfg)
    nc = _CACHED[key]

    # cold first execution of a freshly-loaded NEFF measures ~15-20% slow
    # (device-side warmup); run once untimed, then the traced run
    run_bass_kernel_spmd(nc, in_maps, core_ids=list(range(NCORES)), trace=False)
    try:
        res = run_bass_kernel_spmd(
            nc, in_maps, core_ids=list(range(NCORES)), trace=TRACE
        )
    except (ImportError, ModuleNotFoundError):
        res = run_bass_kernel_spmd(
            nc, in_maps, core_ids=list(range(NCORES)), trace=False
        )
    LAST_EXEC_NS = res.exec_time_ns

    full = np.zeros((B, 5), f32)
    for c in range(NCORES):
        o = np.asarray(res.results[c]["out"])
        for j, g in enumerate(cores[c]):
            full[g] = o[j]
    return np.ascontiguousarray(full.T, dtype=f32)  # (5, 64)
